# revision 11
# baseline (speedup 1.0000x reference)
"""MoE conv layer (8 experts, top-2) on 8 TRN2 NeuronCores.

Strategy: sparse dispatch. Only B*K = 32 (batch, expert) pairs have nonzero
gates; assign 4 pairs (2 batches x 2 experts) to each of the 8 cores.
Gating/top-k/softmax/loss run on host (tiny: [16,8] logits); gate values are
folded into the conv2 weights, so the device kernel is pure conv-gelu-conv
per pair. Host sums the two expert outputs per batch at the end.

Device kernel (per core, SPMD):
  x images held H-padded ([130, 128] rows x cols, contiguous rows so DMA is
  descriptor-cheap); partitions 0:64 hold x, partitions 64:128 hold x
  shifted down one row so two conv taps (dy=-1, dy=0) stack into one K=128
  matmul. Horizontal (dx) taps need no W padding: a dx!=0 tap simply writes
  a 127-column subrange of the PSUM tile (the dx=0 tap opens the
  accumulation group and covers all columns). conv1 = 3 packed K=128
  matmuls + 3 K=64 single-tap matmuls per 4-row tile (singles for the two
  image-half tiles run in different PE row-groups -> concurrent). GELU
  (exact erf LUT) on ScalarE -> H-padded h buffer (bf16). conv2 = 9 taps x
  K=128, M=64 in bf16; the two image-half tiles go to PE col-groups 0/1
  (psum partitions 0:64 / 64:128) -> concurrent. All matmuls in bf16 with
  fp32 PSUM accumulation (~4e-3 rel; fp32r forbids the odd-width partial
  PSUM writes the unpadded-W scheme needs, and bf16 matches its PE rate).
"""

import numpy as np

H = 128
W = 128
C = 64
HID = 128
B = 16
E = 8
TOPK = 2
NCORES = 8
PAIRS_PER_CORE = 4
BATCH_PER_CORE = 2

_compiled = None


def _build_program():
    import concourse.bass as bass
    import concourse.mybir as mybir
    from concourse import bacc
    from concourse.tile import TileContext

    f32 = mybir.dt.float32
    f32r = mybir.dt.float32r
    bf16 = mybir.dt.bfloat16
    GELU = mybir.ActivationFunctionType.Gelu

    nc = bacc.Bacc("TRN2", target_bir_lowering=False, debug=False,
                   num_devices=NCORES)

    x2 = nc.declare_dram_parameter("x2", [BATCH_PER_CORE, C, H, W], bf16,
                                   isOutput=False)
    # weight layouts are k-major so the load DMA is contiguous
    w1T = nc.declare_dram_parameter("w1T", [128, PAIRS_PER_CORE, 3, 128], bf16,
                                    isOutput=False)
    w1sT = nc.declare_dram_parameter("w1sT", [128, PAIRS_PER_CORE, 3, 128],
                                     bf16, isOutput=False)
    w2T = nc.declare_dram_parameter("w2T", [128, PAIRS_PER_CORE, 9, 128], bf16,
                                    isOutput=False)
    y4 = nc.declare_dram_parameter("y4", [PAIRS_PER_CORE, C, H, W], f32,
                                   isOutput=True)

    NG = H // 8  # 16 groups; group g handles 4-row tiles at rows 4g and 64+4g

    # (dx, rhs col range, out col range): dx=0 first -> it opens the PSUM
    # accumulation group and covers all 128 columns; dx=+-1 taps accumulate
    # into a 127-column subrange (the missing column is the zero pad).
    DXS = [(0, 0, 128, 0), (-1, 0, 127, 1), (1, 1, 128, 0)]

    with TileContext(nc) as tc:
        with (
            tc.tile_pool(name="wpool", bufs=1) as wpool,
            tc.tile_pool(name="xpool", bufs=1) as xpool,
            tc.tile_pool(name="hpool", bufs=1) as hpool,
            tc.tile_pool(name="ypool", bufs=3) as ypool,
            tc.tile_pool(name="pspool", bufs=2, space="PSUM") as pspool,
        ):
            # --- weights resident in SBUF for the whole kernel ---
            w1t = wpool.tile([128, PAIRS_PER_CORE, 3, 128], bf16, name="w1t")
            w1st = wpool.tile([128, PAIRS_PER_CORE, 3, 128], bf16, name="w1st")
            w2t = wpool.tile([128, PAIRS_PER_CORE, 9, 128], bf16, name="w2t")
            # w1t first (first matmul needs it), then batch-0 x below, then
            # the rest of the weights -- keeps the critical path short.
            nc.sync.dma_start(out=w1t, in_=w1T[:])

            # --- H-padded image buffers (no W padding; rows contiguous) ---
            # lo (parts 0:64):  row slot i = x row i-1 (slots 0,129 zero)
            # hi (parts 64:128): row slot i = x row i   (slots 128,129 zero)
            xbufs = [xpool.tile([128, H + 2, W], bf16, name=f"xbuf{i}",
                                tag=f"xbuf{i}") for i in range(BATCH_PER_CORE)]
            # hbuf: row slot i = h row i-1 (slots 0,129 zero)
            hbuf = hpool.tile([128, H + 2, W], bf16, name="hbuf")

            # zero pad rows (interiors fully rewritten per batch/pair,
            # pads stay zero afterwards)
            def pad_and_load_x(bi, dma):
                xb = xbufs[bi]
                nc.vector.memset(xb[0:64, 0, :], 0.0)
                nc.vector.memset(xb[0:64, H + 1, :], 0.0)
                nc.vector.memset(xb[64:128, H, :], 0.0)
                nc.vector.memset(xb[64:128, H + 1, :], 0.0)
                # chunk by x-row quarters, ordered so the first A-tile rows
                # (0:32) and first B-tile rows (64:96) land before the rest;
                # conv1 groups then start about half-way into the load.
                # all lo chunks first: their ranges are disjoint, so the
                # issuing sequencer triggers them back-to-back with no
                # waits; each hi chunk then only waits for its lo overlap.
                for k in (0, 2, 1, 3):
                    r0, r1 = 32 * k, 32 * (k + 1)
                    dma.dma_start(out=xb[0:64, r0 + 1:r1 + 1, :],
                                  in_=x2[bi][:, r0:r1, :])
                for k in (0, 2, 1, 3):
                    r0, r1 = 32 * k, 32 * (k + 1)
                    dma.dma_start(out=xb[64:128, r0:r1, :],
                                  in_=x2[bi][:, r0:r1, :])

            nc.vector.memset(hbuf[:, 0, :], 0.0)
            nc.vector.memset(hbuf[:, H + 1, :], 0.0)

            # batch 0 now (sync/HWDGE: starts immediately); batch 1 is
            # issued at the top of pair 1 below, on the gpsimd stream, so
            # it overlaps pair-0/1 compute without competing at startup.
            pad_and_load_x(0, nc.sync)
            nc.sync.dma_start(out=w1st, in_=w1sT[:])
            nc.sync.dma_start(out=w2t, in_=w2T[:])

            # conv2 taps, dx=0 first (opens the accumulation group)
            TAPS = [(dy, dx, cl, ch, ol)
                    for dx, cl, ch, ol in DXS for dy in (-1, 0, 1)]

            for pair in range(PAIRS_PER_CORE):
                xbuf = xbufs[pair // 2]
                if pair == 1:
                    pad_and_load_x(1, nc.gpsimd)

                # ---------------- conv1 + gelu ----------------
                for g in range(NG):
                    rA = 4 * g
                    rB = 64 + 4 * g
                    hA = pspool.tile([128, 4, 128], f32, name=f"hA_{pair}_{g}",
                                     tag="hA")
                    hB = pspool.tile([128, 4, 128], f32, name=f"hB_{pair}_{g}",
                                     tag="hB")
                    # packed K=128: taps (dy=-1 from lo) + (dy=0 from hi)
                    for i, (dx, cl, ch, ol) in enumerate(DXS):
                        n = ch - cl
                        nc.tensor.matmul(hA[:, :, ol:ol + n],
                                         w1t[:, pair, i, :],
                                         xbuf[:, rA:rA + 4, cl:ch],
                                         start=(i == 0), stop=False)
                    for i, (dx, cl, ch, ol) in enumerate(DXS):
                        n = ch - cl
                        nc.tensor.matmul(hB[:, :, ol:ol + n],
                                         w1t[:, pair, i, :],
                                         xbuf[:, rB:rB + 4, cl:ch],
                                         start=(i == 0), stop=False)
                    # single tap dy=+1, K=64. A reads lo rows (row-groups
                    # 0-1), B reads hi rows (row-groups 2-3) -> concurrent.
                    for i, (dx, cl, ch, ol) in enumerate(DXS):
                        n = ch - cl
                        nc.tensor.matmul(hA[:, :, ol:ol + n],
                                         w1st[0:64, pair, i, :],
                                         xbuf[0:64, rA + 2:rA + 6, cl:ch],
                                         start=False, stop=(i == 2))
                        nc.tensor.matmul(hB[:, :, ol:ol + n],
                                         w1st[64:128, pair, i, :],
                                         xbuf[64:128, rB + 1:rB + 5, cl:ch],
                                         start=False, stop=(i == 2))
                    nc.scalar.activation(hbuf[:, rA + 1:rA + 5, :], hA, GELU)
                    nc.scalar.activation(hbuf[:, rB + 1:rB + 5, :], hB, GELU)

                # ---------------- conv2 (bf16, col-group concurrency) ------
                for g in range(NG):
                    rA = 4 * g
                    rB = 64 + 4 * g
                    yps = pspool.tile([128, 4, 128], f32,
                                      name=f"y_{pair}_{g}", tag="y")
                    for t, (dy, dx, cl, ch, ol) in enumerate(TAPS):
                        st = (t == 0)
                        sp = (t == 8)
                        n = ch - cl
                        # tile A -> psum parts 0:64 (col-group 0), tile B ->
                        # 64:128 (col-group 1): hardware-concurrent in bf16
                        nc.tensor.matmul(yps[0:64, :, ol:ol + n],
                                         w2t[:, pair, t, 0:64],
                                         hbuf[:, rA + 1 + dy:rA + 5 + dy,
                                              cl:ch],
                                         start=st, stop=sp)
                        nc.tensor.matmul(yps[64:128, :, ol:ol + n],
                                         w2t[:, pair, t, 64:128],
                                         hbuf[:, rB + 1 + dy:rB + 5 + dy,
                                              cl:ch],
                                         start=st, stop=sp)
                    ysb = ypool.tile([128, 4, 128], f32,
                                     name=f"ysb_{pair}_{g}", tag="ysb")
                    nc.vector.tensor_copy(ysb, yps)
                    nc.sync.dma_start(out=y4[pair][:, rA:rA + 4, :],
                                      in_=ysb[0:64])
                    nc.sync.dma_start(out=y4[pair][:, rB:rB + 4, :],
                                      in_=ysb[64:128])

    nc.compile()
    return nc


def _get_program():
    global _compiled
    if _compiled is None:
        _compiled = _build_program()
    return _compiled


def _cv_squared(v):
    v = v.astype(np.float64)
    if v.size <= 1:
        return 0.0
    return v.var(ddof=1) / (v.mean() ** 2 + 1e-10)


def _host_gating(x, gate_w, gate_b):
    """Mirror the reference gating exactly (fp32 math, top-2, softmax)."""
    avg = x.mean(axis=(2, 3), dtype=np.float32)
    mx = x.max(axis=(2, 3))
    xv = np.concatenate([avg, mx], axis=1).astype(np.float32)
    logits = (xv @ gate_w.T.astype(np.float32) + gate_b).astype(np.float32)
    top_idx = np.argsort(-logits, axis=1, kind="stable")[:, :TOPK]
    top_vals = np.take_along_axis(logits, top_idx, axis=1)
    m = top_vals.max(axis=1, keepdims=True)
    ex = np.exp((top_vals - m).astype(np.float32))
    top_gates = (ex / ex.sum(axis=1, keepdims=True)).astype(np.float32)
    gates = np.zeros((B, E), np.float32)
    gates[np.arange(B)[:, None], top_idx] = top_gates
    return gates, top_idx, top_gates


def kernel(x, w1, w2, gate_w, gate_b):
    import ml_dtypes
    from concourse.bass_utils import run_bass_kernel_spmd

    x = np.ascontiguousarray(x, dtype=np.float32)
    w1 = np.ascontiguousarray(w1, dtype=np.float32)
    w2 = np.ascontiguousarray(w2, dtype=np.float32)
    gate_w = np.ascontiguousarray(gate_w, dtype=np.float32)
    gate_b = np.ascontiguousarray(gate_b, dtype=np.float32)

    gates, top_idx, top_gates = _host_gating(x, gate_w, gate_b)

    importance = gates.sum(0)
    load = (gates > 0).sum(0).astype(np.float32)
    loss = np.float32((_cv_squared(importance) + _cv_squared(load)) * 0.01)

    # pair p = 2*b + j uses expert top_idx[b, j] with gate top_gates[b, j]
    experts = top_idx.reshape(-1)          # [32]
    gvals = top_gates.reshape(-1)          # [32]

    # conv1 weights, pre-transposed to [K, pair, dx, M] (k-major for DMA).
    # packed rows 0:64 = tap dy=-1, rows 64:128 = tap dy=0; singles = dy=+1
    # duplicated in both halves. dx slot order matches DXS: (0, -1, +1).
    dxorder = [1, 0, 2]  # dx index (0->dx=-1,1->dx=0,2->dx=+1) in DXS order
    w1sel = w1[experts]                    # [32, 128, 64, 3, 3]
    w1Tg = np.empty((B * TOPK, 128, 3, 128), np.float32)
    w1Tg[:, 0:64] = np.transpose(w1sel[:, :, :, 0, :], (0, 2, 3, 1))[:, :, dxorder]
    w1Tg[:, 64:128] = np.transpose(w1sel[:, :, :, 1, :], (0, 2, 3, 1))[:, :, dxorder]
    w1sTg = np.empty((B * TOPK, 128, 3, 128), np.float32)
    tap2 = np.transpose(w1sel[:, :, :, 2, :], (0, 2, 3, 1))[:, :, dxorder]
    w1sTg[:, 0:64] = tap2
    w1sTg[:, 64:128] = tap2

    # conv2 weights with gate folded in, duplicated along M for col-tiling.
    # tap order must match TAPS: dx in (0,-1,+1) outer, dy in (-1,0,1) inner.
    w2sel = w2[experts] * gvals[:, None, None, None, None]  # [32,64,128,3,3]
    w2r = np.transpose(w2sel, (0, 2, 4, 3, 1))  # [p, k, dx, dy, m]
    w2r = w2r[:, :, [1, 0, 2], :, :].reshape(B * TOPK, 128, 9, 64)
    w2Tg = np.concatenate([w2r, w2r], axis=3)   # [32, 128, 9, 128]

    nc = _get_program()
    in_maps = []
    for c in range(NCORES):
        p0 = PAIRS_PER_CORE * c
        sl = slice(p0, p0 + PAIRS_PER_CORE)
        in_maps.append({
            "x2": np.ascontiguousarray(
                x[BATCH_PER_CORE * c: BATCH_PER_CORE * (c + 1)]
            ).astype(ml_dtypes.bfloat16),
            "w1T": np.ascontiguousarray(
                w1Tg[sl].transpose(1, 0, 2, 3)).astype(ml_dtypes.bfloat16),
            "w1sT": np.ascontiguousarray(
                w1sTg[sl].transpose(1, 0, 2, 3)).astype(ml_dtypes.bfloat16),
            "w2T": np.ascontiguousarray(
                w2Tg[sl].transpose(1, 0, 2, 3)).astype(ml_dtypes.bfloat16),
        })

    res = run_bass_kernel_spmd(nc, in_maps, core_ids=list(range(NCORES)),
                               **_RUN_KWARGS)
    _LAST_RESULT[0] = res

    y = np.empty((B, C, H, W), np.float32)
    for c in range(NCORES):
        y4c = res.results[c]["y4"]
        y[2 * c] = y4c[0] + y4c[1]
        y[2 * c + 1] = y4c[2] + y4c[3]
    return y, loss


# test-harness knobs (kernel.py stays self-contained; defaults are no-ops)
_RUN_KWARGS = {}
_LAST_RESULT = [None]


# revision 12
# speedup vs baseline: 1.0012x; 1.0012x over previous
"""MoE conv layer (8 experts, top-2) on 8 TRN2 NeuronCores.

Strategy: sparse dispatch. Only B*K = 32 (batch, expert) pairs have nonzero
gates; assign 4 pairs (2 batches x 2 experts) to each of the 8 cores.
Gating/top-k/softmax/loss run on host (tiny: [16,8] logits); gate values are
folded into the conv2 weights, so the device kernel is pure conv-gelu-conv
per pair. Host sums the two expert outputs per batch at the end.

Device kernel (per core, SPMD):
  x images held H-padded ([130, 128] rows x cols, contiguous rows so DMA is
  descriptor-cheap); partitions 0:64 hold x, partitions 64:128 hold x
  shifted down one row so two conv taps (dy=-1, dy=0) stack into one K=128
  matmul. Horizontal (dx) taps need no W padding: a dx!=0 tap simply writes
  a 127-column subrange of the PSUM tile (the dx=0 tap opens the
  accumulation group and covers all columns). conv1 = 3 packed K=128
  matmuls + 3 K=64 single-tap matmuls per 4-row tile (singles for the two
  image-half tiles run in different PE row-groups -> concurrent). GELU
  (exact erf LUT) on ScalarE -> H-padded h buffer (bf16). conv2 = 9 taps x
  K=128, M=64 in bf16; the two image-half tiles go to PE col-groups 0/1
  (psum partitions 0:64 / 64:128) -> concurrent. All matmuls in bf16 with
  fp32 PSUM accumulation (~4e-3 rel; fp32r forbids the odd-width partial
  PSUM writes the unpadded-W scheme needs, and bf16 matches its PE rate).
"""

import numpy as np

H = 128
W = 128
C = 64
HID = 128
B = 16
E = 8
TOPK = 2
NCORES = 8
PAIRS_PER_CORE = 4
BATCH_PER_CORE = 2

_compiled = None


def _build_program():
    import concourse.bass as bass
    import concourse.mybir as mybir
    from concourse import bacc
    from concourse.tile import TileContext

    f32 = mybir.dt.float32
    f32r = mybir.dt.float32r
    bf16 = mybir.dt.bfloat16
    GELU = mybir.ActivationFunctionType.Gelu

    nc = bacc.Bacc("TRN2", target_bir_lowering=False, debug=False,
                   num_devices=NCORES)

    x2 = nc.declare_dram_parameter("x2", [BATCH_PER_CORE, C, H, W], bf16,
                                   isOutput=False)
    # weight layouts are k-major so the load DMA is contiguous
    w1T = nc.declare_dram_parameter("w1T", [128, PAIRS_PER_CORE, 3, 128], bf16,
                                    isOutput=False)
    w1sT = nc.declare_dram_parameter("w1sT", [128, PAIRS_PER_CORE, 3, 128],
                                     bf16, isOutput=False)
    w2T = nc.declare_dram_parameter("w2T", [128, PAIRS_PER_CORE, 9, 128], bf16,
                                    isOutput=False)
    y4 = nc.declare_dram_parameter("y4", [PAIRS_PER_CORE, C, H, W], f32,
                                   isOutput=True)

    NG = H // 8  # 16 groups; group g handles 4-row tiles at rows 4g and 64+4g

    # (dx, rhs col range, out col range): dx=0 first -> it opens the PSUM
    # accumulation group and covers all 128 columns; dx=+-1 taps accumulate
    # into a 127-column subrange (the missing column is the zero pad).
    DXS = [(0, 0, 128, 0), (-1, 0, 127, 1), (1, 1, 128, 0)]

    with TileContext(nc) as tc:
        with (
            tc.tile_pool(name="wpool", bufs=1) as wpool,
            tc.tile_pool(name="xpool", bufs=1) as xpool,
            tc.tile_pool(name="hpool", bufs=1) as hpool,
            tc.tile_pool(name="ypool", bufs=3) as ypool,
            tc.tile_pool(name="pspool", bufs=2, space="PSUM") as pspool,
        ):
            # --- weights resident in SBUF for the whole kernel ---
            w1t = wpool.tile([128, PAIRS_PER_CORE, 3, 128], bf16, name="w1t")
            w1st = wpool.tile([128, PAIRS_PER_CORE, 3, 128], bf16, name="w1st")
            w2t = wpool.tile([128, PAIRS_PER_CORE, 9, 128], bf16, name="w2t")
            # w1t first (first matmul needs it), then batch-0 x below, then
            # the rest of the weights -- keeps the critical path short.
            nc.sync.dma_start(out=w1t, in_=w1T[:])

            # --- H-padded image buffers (no W padding; rows contiguous) ---
            # lo (parts 0:64):  row slot i = x row i-1 (slots 0,129 zero)
            # hi (parts 64:128): row slot i = x row i   (slots 128,129 zero)
            xbufs = [xpool.tile([128, H + 2, W], bf16, name=f"xbuf{i}",
                                tag=f"xbuf{i}") for i in range(BATCH_PER_CORE)]
            # hbuf: row slot i = h row i-1 (slots 0,129 zero)
            hbuf = hpool.tile([128, H + 2, W], bf16, name="hbuf")

            # zero pad rows (interiors fully rewritten per batch/pair,
            # pads stay zero afterwards)
            def pad_and_load_x(bi, dma):
                xb = xbufs[bi]
                nc.vector.memset(xb[0:64, 0, :], 0.0)
                nc.vector.memset(xb[0:64, H + 1, :], 0.0)
                nc.vector.memset(xb[64:128, H, :], 0.0)
                nc.vector.memset(xb[64:128, H + 1, :], 0.0)
                # chunk by x-row quarters, ordered so the first A-tile rows
                # (0:32) and first B-tile rows (64:96) land before the rest;
                # conv1 groups then start about half-way into the load.
                for k in (0, 2, 1, 3):
                    r0, r1 = 32 * k, 32 * (k + 1)
                    dma.dma_start(out=xb[0:64, r0 + 1:r1 + 1, :],
                                  in_=x2[bi][:, r0:r1, :])
                    dma.dma_start(out=xb[64:128, r0:r1, :],
                                  in_=x2[bi][:, r0:r1, :])

            nc.vector.memset(hbuf[:, 0, :], 0.0)
            nc.vector.memset(hbuf[:, H + 1, :], 0.0)

            # batch 0 now (sync/HWDGE: starts immediately); batch 1 is
            # issued at the top of pair 1 below, on the gpsimd stream, so
            # it overlaps pair-0/1 compute without competing at startup.
            pad_and_load_x(0, nc.sync)
            nc.sync.dma_start(out=w1st, in_=w1sT[:])
            nc.sync.dma_start(out=w2t, in_=w2T[:])

            # conv2 taps, dx=0 first (opens the accumulation group)
            TAPS = [(dy, dx, cl, ch, ol)
                    for dx, cl, ch, ol in DXS for dy in (-1, 0, 1)]

            for pair in range(PAIRS_PER_CORE):
                xbuf = xbufs[pair // 2]
                if pair == 1:
                    pad_and_load_x(1, nc.gpsimd)

                # ---------------- conv1 + gelu ----------------
                for g in range(NG):
                    rA = 4 * g
                    rB = 64 + 4 * g
                    hA = pspool.tile([128, 4, 128], f32, name=f"hA_{pair}_{g}",
                                     tag="hA")
                    hB = pspool.tile([128, 4, 128], f32, name=f"hB_{pair}_{g}",
                                     tag="hB")
                    # packed K=128: taps (dy=-1 from lo) + (dy=0 from hi)
                    for i, (dx, cl, ch, ol) in enumerate(DXS):
                        n = ch - cl
                        nc.tensor.matmul(hA[:, :, ol:ol + n],
                                         w1t[:, pair, i, :],
                                         xbuf[:, rA:rA + 4, cl:ch],
                                         start=(i == 0), stop=False)
                    for i, (dx, cl, ch, ol) in enumerate(DXS):
                        n = ch - cl
                        nc.tensor.matmul(hB[:, :, ol:ol + n],
                                         w1t[:, pair, i, :],
                                         xbuf[:, rB:rB + 4, cl:ch],
                                         start=(i == 0), stop=False)
                    # single tap dy=+1, K=64. A reads lo rows (row-groups
                    # 0-1), B reads hi rows (row-groups 2-3) -> concurrent.
                    for i, (dx, cl, ch, ol) in enumerate(DXS):
                        n = ch - cl
                        nc.tensor.matmul(hA[:, :, ol:ol + n],
                                         w1st[0:64, pair, i, :],
                                         xbuf[0:64, rA + 2:rA + 6, cl:ch],
                                         start=False, stop=(i == 2))
                        nc.tensor.matmul(hB[:, :, ol:ol + n],
                                         w1st[64:128, pair, i, :],
                                         xbuf[64:128, rB + 1:rB + 5, cl:ch],
                                         start=False, stop=(i == 2))
                    nc.scalar.activation(hbuf[:, rA + 1:rA + 5, :], hA, GELU)
                    nc.scalar.activation(hbuf[:, rB + 1:rB + 5, :], hB, GELU)

                # ---------------- conv2 (bf16, col-group concurrency) ------
                for g in range(NG):
                    rA = 4 * g
                    rB = 64 + 4 * g
                    yps = pspool.tile([128, 4, 128], f32,
                                      name=f"y_{pair}_{g}", tag="y")
                    for t, (dy, dx, cl, ch, ol) in enumerate(TAPS):
                        st = (t == 0)
                        sp = (t == 8)
                        n = ch - cl
                        # tile A -> psum parts 0:64 (col-group 0), tile B ->
                        # 64:128 (col-group 1): hardware-concurrent in bf16
                        nc.tensor.matmul(yps[0:64, :, ol:ol + n],
                                         w2t[:, pair, t, 0:64],
                                         hbuf[:, rA + 1 + dy:rA + 5 + dy,
                                              cl:ch],
                                         start=st, stop=sp)
                        nc.tensor.matmul(yps[64:128, :, ol:ol + n],
                                         w2t[:, pair, t, 64:128],
                                         hbuf[:, rB + 1 + dy:rB + 5 + dy,
                                              cl:ch],
                                         start=st, stop=sp)
                    ysb = ypool.tile([128, 4, 128], f32,
                                     name=f"ysb_{pair}_{g}", tag="ysb")
                    nc.vector.tensor_copy(ysb, yps)
                    nc.sync.dma_start(out=y4[pair][:, rA:rA + 4, :],
                                      in_=ysb[0:64])
                    nc.sync.dma_start(out=y4[pair][:, rB:rB + 4, :],
                                      in_=ysb[64:128])

    nc.compile()
    return nc


def _get_program():
    global _compiled
    if _compiled is None:
        _compiled = _build_program()
    return _compiled


def _cv_squared(v):
    v = v.astype(np.float64)
    if v.size <= 1:
        return 0.0
    return v.var(ddof=1) / (v.mean() ** 2 + 1e-10)


def _host_gating(x, gate_w, gate_b):
    """Mirror the reference gating exactly (fp32 math, top-2, softmax)."""
    avg = x.mean(axis=(2, 3), dtype=np.float32)
    mx = x.max(axis=(2, 3))
    xv = np.concatenate([avg, mx], axis=1).astype(np.float32)
    logits = (xv @ gate_w.T.astype(np.float32) + gate_b).astype(np.float32)
    top_idx = np.argsort(-logits, axis=1, kind="stable")[:, :TOPK]
    top_vals = np.take_along_axis(logits, top_idx, axis=1)
    m = top_vals.max(axis=1, keepdims=True)
    ex = np.exp((top_vals - m).astype(np.float32))
    top_gates = (ex / ex.sum(axis=1, keepdims=True)).astype(np.float32)
    gates = np.zeros((B, E), np.float32)
    gates[np.arange(B)[:, None], top_idx] = top_gates
    return gates, top_idx, top_gates


def kernel(x, w1, w2, gate_w, gate_b):
    import ml_dtypes
    from concourse.bass_utils import run_bass_kernel_spmd

    x = np.ascontiguousarray(x, dtype=np.float32)
    w1 = np.ascontiguousarray(w1, dtype=np.float32)
    w2 = np.ascontiguousarray(w2, dtype=np.float32)
    gate_w = np.ascontiguousarray(gate_w, dtype=np.float32)
    gate_b = np.ascontiguousarray(gate_b, dtype=np.float32)

    gates, top_idx, top_gates = _host_gating(x, gate_w, gate_b)

    importance = gates.sum(0)
    load = (gates > 0).sum(0).astype(np.float32)
    loss = np.float32((_cv_squared(importance) + _cv_squared(load)) * 0.01)

    # pair p = 2*b + j uses expert top_idx[b, j] with gate top_gates[b, j]
    experts = top_idx.reshape(-1)          # [32]
    gvals = top_gates.reshape(-1)          # [32]

    # conv1 weights, pre-transposed to [K, pair, dx, M] (k-major for DMA).
    # packed rows 0:64 = tap dy=-1, rows 64:128 = tap dy=0; singles = dy=+1
    # duplicated in both halves. dx slot order matches DXS: (0, -1, +1).
    dxorder = [1, 0, 2]  # dx index (0->dx=-1,1->dx=0,2->dx=+1) in DXS order
    w1sel = w1[experts]                    # [32, 128, 64, 3, 3]
    w1Tg = np.empty((B * TOPK, 128, 3, 128), np.float32)
    w1Tg[:, 0:64] = np.transpose(w1sel[:, :, :, 0, :], (0, 2, 3, 1))[:, :, dxorder]
    w1Tg[:, 64:128] = np.transpose(w1sel[:, :, :, 1, :], (0, 2, 3, 1))[:, :, dxorder]
    w1sTg = np.empty((B * TOPK, 128, 3, 128), np.float32)
    tap2 = np.transpose(w1sel[:, :, :, 2, :], (0, 2, 3, 1))[:, :, dxorder]
    w1sTg[:, 0:64] = tap2
    w1sTg[:, 64:128] = tap2

    # conv2 weights with gate folded in, duplicated along M for col-tiling.
    # tap order must match TAPS: dx in (0,-1,+1) outer, dy in (-1,0,1) inner.
    w2sel = w2[experts] * gvals[:, None, None, None, None]  # [32,64,128,3,3]
    w2r = np.transpose(w2sel, (0, 2, 4, 3, 1))  # [p, k, dx, dy, m]
    w2r = w2r[:, :, [1, 0, 2], :, :].reshape(B * TOPK, 128, 9, 64)
    w2Tg = np.concatenate([w2r, w2r], axis=3)   # [32, 128, 9, 128]

    nc = _get_program()
    in_maps = []
    for c in range(NCORES):
        p0 = PAIRS_PER_CORE * c
        sl = slice(p0, p0 + PAIRS_PER_CORE)
        in_maps.append({
            "x2": np.ascontiguousarray(
                x[BATCH_PER_CORE * c: BATCH_PER_CORE * (c + 1)]
            ).astype(ml_dtypes.bfloat16),
            "w1T": np.ascontiguousarray(
                w1Tg[sl].transpose(1, 0, 2, 3)).astype(ml_dtypes.bfloat16),
            "w1sT": np.ascontiguousarray(
                w1sTg[sl].transpose(1, 0, 2, 3)).astype(ml_dtypes.bfloat16),
            "w2T": np.ascontiguousarray(
                w2Tg[sl].transpose(1, 0, 2, 3)).astype(ml_dtypes.bfloat16),
        })

    res = run_bass_kernel_spmd(nc, in_maps, core_ids=list(range(NCORES)),
                               **_RUN_KWARGS)
    _LAST_RESULT[0] = res

    y = np.empty((B, C, H, W), np.float32)
    for c in range(NCORES):
        y4c = res.results[c]["y4"]
        y[2 * c] = y4c[0] + y4c[1]
        y[2 * c + 1] = y4c[2] + y4c[3]
    return y, loss


# test-harness knobs (kernel.py stays self-contained; defaults are no-ops)
_RUN_KWARGS = {}
_LAST_RESULT = [None]


# revision 13
# speedup vs baseline: 1.0569x; 1.0556x over previous
"""MoE conv layer (8 experts, top-2) on 8 TRN2 NeuronCores.

Strategy: sparse dispatch. Only B*K = 32 (batch, expert) pairs have nonzero
gates; assign 4 pairs (2 batches x 2 experts) to each of the 8 cores.
Gating/top-k/softmax/loss run on host (tiny: [16,8] logits); gate values are
folded into the conv2 weights, so the device kernel is pure conv-gelu-conv
per pair. Host sums the two expert outputs per batch at the end.

Device kernel (per core, SPMD):
  x images held H-padded ([130, 128] rows x cols, contiguous rows so DMA is
  descriptor-cheap); partitions 0:64 hold x, partitions 64:128 hold x
  shifted down one row so two conv taps (dy=-1, dy=0) stack into one K=128
  matmul. Horizontal (dx) taps need no W padding: a dx!=0 tap simply writes
  a 127-column subrange of the PSUM tile (the dx=0 tap opens the
  accumulation group and covers all columns). conv1 = 3 packed K=128
  matmuls + 3 K=64 single-tap matmuls per 4-row tile (singles for the two
  image-half tiles run in different PE row-groups -> concurrent). GELU
  (exact erf LUT) on ScalarE -> H-padded h buffer (bf16). conv2 = 9 taps x
  K=128, M=64 in bf16; the two image-half tiles go to PE col-groups 0/1
  (psum partitions 0:64 / 64:128) -> concurrent. All matmuls in bf16 with
  fp32 PSUM accumulation (~4e-3 rel; fp32r forbids the odd-width partial
  PSUM writes the unpadded-W scheme needs, and bf16 matches its PE rate).
"""

import numpy as np

H = 128
W = 128
C = 64
HID = 128
B = 16
E = 8
TOPK = 2
NCORES = 8
PAIRS_PER_CORE = 4
BATCH_PER_CORE = 2

_compiled = None


def _build_program():
    import concourse.bass as bass
    import concourse.mybir as mybir
    from concourse import bacc
    from concourse.tile import TileContext

    f32 = mybir.dt.float32
    f32r = mybir.dt.float32r
    bf16 = mybir.dt.bfloat16
    GELU = mybir.ActivationFunctionType.Gelu

    nc = bacc.Bacc("TRN2", target_bir_lowering=False, debug=False,
                   num_devices=NCORES)

    x2 = nc.declare_dram_parameter("x2", [BATCH_PER_CORE, C, H, W], bf16,
                                   isOutput=False)
    # weight layouts are k-major so the load DMA is contiguous
    w1T = nc.declare_dram_parameter("w1T", [128, PAIRS_PER_CORE, 3, 128], bf16,
                                    isOutput=False)
    w1sT = nc.declare_dram_parameter("w1sT", [128, PAIRS_PER_CORE, 3, 128],
                                     bf16, isOutput=False)
    w2T = nc.declare_dram_parameter("w2T", [128, PAIRS_PER_CORE, 9, 128], bf16,
                                    isOutput=False)
    y4 = nc.declare_dram_parameter("y4", [PAIRS_PER_CORE, C, H, W], f32,
                                   isOutput=True)

    NG = H // 8  # 16 groups; group g handles 4-row tiles at rows 4g and 64+4g

    # (dx, rhs col range, out col range): dx=0 first -> it opens the PSUM
    # accumulation group and covers all 128 columns; dx=+-1 taps accumulate
    # into a 127-column subrange (the missing column is the zero pad).
    DXS = [(0, 0, 128, 0), (-1, 0, 127, 1), (1, 1, 128, 0)]

    with TileContext(nc) as tc:
        with (
            tc.tile_pool(name="wpool", bufs=1) as wpool,
            tc.tile_pool(name="xpool", bufs=1) as xpool,
            tc.tile_pool(name="hpool", bufs=1) as hpool,
            tc.tile_pool(name="ypool", bufs=3) as ypool,
            tc.tile_pool(name="pspool", bufs=2, space="PSUM") as pspool,
        ):
            # --- weights resident in SBUF for the whole kernel ---
            w1t = wpool.tile([128, PAIRS_PER_CORE, 3, 128], bf16, name="w1t")
            w1st = wpool.tile([128, PAIRS_PER_CORE, 3, 128], bf16, name="w1st")
            w2t = wpool.tile([128, PAIRS_PER_CORE, 9, 128], bf16, name="w2t")
            # w1t first (first matmul needs it), then batch-0 x below, then
            # the rest of the weights -- keeps the critical path short.
            nc.sync.dma_start(out=w1t, in_=w1T[:])

            # --- H-padded image buffers (no W padding; rows contiguous) ---
            # lo (parts 0:64):  row slot i = x row i-1 (slots 0,129 zero)
            # hi (parts 64:128): row slot i = x row i   (slots 128,129 zero)
            xbufs = [xpool.tile([128, H + 2, W], bf16, name=f"xbuf{i}",
                                tag=f"xbuf{i}") for i in range(BATCH_PER_CORE)]
            # hbuf: row slot i = h row i-1 (slots 0,129 zero)
            hbuf = hpool.tile([128, H + 2, W], bf16, name="hbuf")

            # zero pad rows (interiors fully rewritten per batch/pair,
            # pads stay zero afterwards)
            def pad_and_load_x(bi, dma_lo, dma_hi):
                xb = xbufs[bi]
                nc.vector.memset(xb[0:64, 0, :], 0.0)
                nc.vector.memset(xb[0:64, H + 1, :], 0.0)
                nc.vector.memset(xb[64:128, H, :], 0.0)
                nc.vector.memset(xb[64:128, H + 1, :], 0.0)
                # chunk by x-row quarters, ordered so the first A-tile rows
                # (0:32) and first B-tile rows (64:96) land before the rest.
                # Each hi chunk byte-overlaps its lo chunk, so its wait
                # head-of-line-blocks the issuing stream -- put lo and hi on
                # DIFFERENT streams so all lo chunks trigger back-to-back.
                for k in (0, 2, 1, 3):
                    r0, r1 = 32 * k, 32 * (k + 1)
                    dma_lo.dma_start(out=xb[0:64, r0 + 1:r1 + 1, :],
                                     in_=x2[bi][:, r0:r1, :])
                for k in (0, 2, 1, 3):
                    r0, r1 = 32 * k, 32 * (k + 1)
                    dma_hi.dma_start(out=xb[64:128, r0:r1, :],
                                     in_=x2[bi][:, r0:r1, :])

            nc.vector.memset(hbuf[:, 0, :], 0.0)
            nc.vector.memset(hbuf[:, H + 1, :], 0.0)

            # batch 0 now (sync/HWDGE: starts immediately); batch 1 is
            # issued at the top of pair 1 below, on the gpsimd stream, so
            # it overlaps pair-0/1 compute without competing at startup.
            pad_and_load_x(0, nc.sync, nc.gpsimd)

            # warm the PE (HAM clock gate) with throwaway matmuls while the
            # x load streams in; they only depend on the (tiny) w1t DMA.
            warm = pspool.tile([128, 3, 128], f32, name="warm", tag="warm",
                               bufs=1)
            for _ in range(30):
                nc.tensor.matmul(warm, w1t[:, 0, 0, :], w1t[:, 0, :, :],
                                 start=True, stop=True)
            nc.sync.dma_start(out=w1st, in_=w1sT[:])
            nc.sync.dma_start(out=w2t, in_=w2T[:])

            # conv2 taps, dx=0 first (opens the accumulation group)
            TAPS = [(dy, dx, cl, ch, ol)
                    for dx, cl, ch, ol in DXS for dy in (-1, 0, 1)]

            for pair in range(PAIRS_PER_CORE):
                xbuf = xbufs[pair // 2]
                if pair == 1:
                    pad_and_load_x(1, nc.gpsimd, nc.gpsimd)

                # ---------------- conv1 + gelu ----------------
                for g in range(NG):
                    rA = 4 * g
                    rB = 64 + 4 * g
                    hA = pspool.tile([128, 4, 128], f32, name=f"hA_{pair}_{g}",
                                     tag="hA")
                    hB = pspool.tile([128, 4, 128], f32, name=f"hB_{pair}_{g}",
                                     tag="hB")
                    # packed K=128: taps (dy=-1 from lo) + (dy=0 from hi)
                    for i, (dx, cl, ch, ol) in enumerate(DXS):
                        n = ch - cl
                        nc.tensor.matmul(hA[:, :, ol:ol + n],
                                         w1t[:, pair, i, :],
                                         xbuf[:, rA:rA + 4, cl:ch],
                                         start=(i == 0), stop=False)
                    for i, (dx, cl, ch, ol) in enumerate(DXS):
                        n = ch - cl
                        nc.tensor.matmul(hB[:, :, ol:ol + n],
                                         w1t[:, pair, i, :],
                                         xbuf[:, rB:rB + 4, cl:ch],
                                         start=(i == 0), stop=False)
                    # single tap dy=+1, K=64. A reads lo rows (row-groups
                    # 0-1), B reads hi rows (row-groups 2-3) -> concurrent.
                    for i, (dx, cl, ch, ol) in enumerate(DXS):
                        n = ch - cl
                        nc.tensor.matmul(hA[:, :, ol:ol + n],
                                         w1st[0:64, pair, i, :],
                                         xbuf[0:64, rA + 2:rA + 6, cl:ch],
                                         start=False, stop=(i == 2))
                        nc.tensor.matmul(hB[:, :, ol:ol + n],
                                         w1st[64:128, pair, i, :],
                                         xbuf[64:128, rB + 1:rB + 5, cl:ch],
                                         start=False, stop=(i == 2))
                    nc.scalar.activation(hbuf[:, rA + 1:rA + 5, :], hA, GELU)
                    nc.scalar.activation(hbuf[:, rB + 1:rB + 5, :], hB, GELU)

                # ---------------- conv2 (bf16, col-group concurrency) ------
                for g in range(NG):
                    rA = 4 * g
                    rB = 64 + 4 * g
                    yps = pspool.tile([128, 4, 128], f32,
                                      name=f"y_{pair}_{g}", tag="y")
                    for t, (dy, dx, cl, ch, ol) in enumerate(TAPS):
                        st = (t == 0)
                        sp = (t == 8)
                        n = ch - cl
                        # tile A -> psum parts 0:64 (col-group 0), tile B ->
                        # 64:128 (col-group 1): hardware-concurrent in bf16
                        nc.tensor.matmul(yps[0:64, :, ol:ol + n],
                                         w2t[:, pair, t, 0:64],
                                         hbuf[:, rA + 1 + dy:rA + 5 + dy,
                                              cl:ch],
                                         start=st, stop=sp)
                        nc.tensor.matmul(yps[64:128, :, ol:ol + n],
                                         w2t[:, pair, t, 64:128],
                                         hbuf[:, rB + 1 + dy:rB + 5 + dy,
                                              cl:ch],
                                         start=st, stop=sp)
                    ysb = ypool.tile([128, 4, 128], f32,
                                     name=f"ysb_{pair}_{g}", tag="ysb")
                    nc.vector.tensor_copy(ysb, yps)
                    nc.sync.dma_start(out=y4[pair][:, rA:rA + 4, :],
                                      in_=ysb[0:64])
                    nc.sync.dma_start(out=y4[pair][:, rB:rB + 4, :],
                                      in_=ysb[64:128])

    nc.compile()
    return nc


def _get_program():
    global _compiled
    if _compiled is None:
        _compiled = _build_program()
    return _compiled


def _cv_squared(v):
    v = v.astype(np.float64)
    if v.size <= 1:
        return 0.0
    return v.var(ddof=1) / (v.mean() ** 2 + 1e-10)


def _host_gating(x, gate_w, gate_b):
    """Mirror the reference gating exactly (fp32 math, top-2, softmax)."""
    avg = x.mean(axis=(2, 3), dtype=np.float32)
    mx = x.max(axis=(2, 3))
    xv = np.concatenate([avg, mx], axis=1).astype(np.float32)
    logits = (xv @ gate_w.T.astype(np.float32) + gate_b).astype(np.float32)
    top_idx = np.argsort(-logits, axis=1, kind="stable")[:, :TOPK]
    top_vals = np.take_along_axis(logits, top_idx, axis=1)
    m = top_vals.max(axis=1, keepdims=True)
    ex = np.exp((top_vals - m).astype(np.float32))
    top_gates = (ex / ex.sum(axis=1, keepdims=True)).astype(np.float32)
    gates = np.zeros((B, E), np.float32)
    gates[np.arange(B)[:, None], top_idx] = top_gates
    return gates, top_idx, top_gates


def kernel(x, w1, w2, gate_w, gate_b):
    import ml_dtypes
    from concourse.bass_utils import run_bass_kernel_spmd

    x = np.ascontiguousarray(x, dtype=np.float32)
    w1 = np.ascontiguousarray(w1, dtype=np.float32)
    w2 = np.ascontiguousarray(w2, dtype=np.float32)
    gate_w = np.ascontiguousarray(gate_w, dtype=np.float32)
    gate_b = np.ascontiguousarray(gate_b, dtype=np.float32)

    gates, top_idx, top_gates = _host_gating(x, gate_w, gate_b)

    importance = gates.sum(0)
    load = (gates > 0).sum(0).astype(np.float32)
    loss = np.float32((_cv_squared(importance) + _cv_squared(load)) * 0.01)

    # pair p = 2*b + j uses expert top_idx[b, j] with gate top_gates[b, j]
    experts = top_idx.reshape(-1)          # [32]
    gvals = top_gates.reshape(-1)          # [32]

    # conv1 weights, pre-transposed to [K, pair, dx, M] (k-major for DMA).
    # packed rows 0:64 = tap dy=-1, rows 64:128 = tap dy=0; singles = dy=+1
    # duplicated in both halves. dx slot order matches DXS: (0, -1, +1).
    dxorder = [1, 0, 2]  # dx index (0->dx=-1,1->dx=0,2->dx=+1) in DXS order
    w1sel = w1[experts]                    # [32, 128, 64, 3, 3]
    w1Tg = np.empty((B * TOPK, 128, 3, 128), np.float32)
    w1Tg[:, 0:64] = np.transpose(w1sel[:, :, :, 0, :], (0, 2, 3, 1))[:, :, dxorder]
    w1Tg[:, 64:128] = np.transpose(w1sel[:, :, :, 1, :], (0, 2, 3, 1))[:, :, dxorder]
    w1sTg = np.empty((B * TOPK, 128, 3, 128), np.float32)
    tap2 = np.transpose(w1sel[:, :, :, 2, :], (0, 2, 3, 1))[:, :, dxorder]
    w1sTg[:, 0:64] = tap2
    w1sTg[:, 64:128] = tap2

    # conv2 weights with gate folded in, duplicated along M for col-tiling.
    # tap order must match TAPS: dx in (0,-1,+1) outer, dy in (-1,0,1) inner.
    w2sel = w2[experts] * gvals[:, None, None, None, None]  # [32,64,128,3,3]
    w2r = np.transpose(w2sel, (0, 2, 4, 3, 1))  # [p, k, dx, dy, m]
    w2r = w2r[:, :, [1, 0, 2], :, :].reshape(B * TOPK, 128, 9, 64)
    w2Tg = np.concatenate([w2r, w2r], axis=3)   # [32, 128, 9, 128]

    nc = _get_program()
    in_maps = []
    for c in range(NCORES):
        p0 = PAIRS_PER_CORE * c
        sl = slice(p0, p0 + PAIRS_PER_CORE)
        in_maps.append({
            "x2": np.ascontiguousarray(
                x[BATCH_PER_CORE * c: BATCH_PER_CORE * (c + 1)]
            ).astype(ml_dtypes.bfloat16),
            "w1T": np.ascontiguousarray(
                w1Tg[sl].transpose(1, 0, 2, 3)).astype(ml_dtypes.bfloat16),
            "w1sT": np.ascontiguousarray(
                w1sTg[sl].transpose(1, 0, 2, 3)).astype(ml_dtypes.bfloat16),
            "w2T": np.ascontiguousarray(
                w2Tg[sl].transpose(1, 0, 2, 3)).astype(ml_dtypes.bfloat16),
        })

    res = run_bass_kernel_spmd(nc, in_maps, core_ids=list(range(NCORES)),
                               **_RUN_KWARGS)
    _LAST_RESULT[0] = res

    y = np.empty((B, C, H, W), np.float32)
    for c in range(NCORES):
        y4c = res.results[c]["y4"]
        y[2 * c] = y4c[0] + y4c[1]
        y[2 * c + 1] = y4c[2] + y4c[3]
    return y, loss


# test-harness knobs (kernel.py stays self-contained; defaults are no-ops)
_RUN_KWARGS = {}
_LAST_RESULT = [None]


# revision 14
# speedup vs baseline: 1.0723x; 1.0146x over previous
"""MoE conv layer (8 experts, top-2) on 8 TRN2 NeuronCores.

Strategy: sparse dispatch. Only B*K = 32 (batch, expert) pairs have nonzero
gates; assign 4 pairs (2 batches x 2 experts) to each of the 8 cores.
Gating/top-k/softmax/loss run on host (tiny: [16,8] logits); gate values are
folded into the conv2 weights, so the device kernel is pure conv-gelu-conv
per pair. Host sums the two expert outputs per batch at the end.

Device kernel (per core, SPMD):
  x images held H-padded ([130, 128] rows x cols, contiguous rows so DMA is
  descriptor-cheap); partitions 0:64 hold x, partitions 64:128 hold x
  shifted down one row so two conv taps (dy=-1, dy=0) stack into one K=128
  matmul. Horizontal (dx) taps need no W padding: a dx!=0 tap simply writes
  a 127-column subrange of the PSUM tile (the dx=0 tap opens the
  accumulation group and covers all columns). conv1 = 3 packed K=128
  matmuls + 3 K=64 single-tap matmuls per 4-row tile (singles for the two
  image-half tiles run in different PE row-groups -> concurrent). GELU
  (exact erf LUT) on ScalarE -> H-padded h buffer (bf16). conv2 = 9 taps x
  K=128, M=64 in bf16; the two image-half tiles go to PE col-groups 0/1
  (psum partitions 0:64 / 64:128) -> concurrent. All matmuls in bf16 with
  fp32 PSUM accumulation (~4e-3 rel; fp32r forbids the odd-width partial
  PSUM writes the unpadded-W scheme needs, and bf16 matches its PE rate).
"""

import numpy as np

H = 128
W = 128
C = 64
HID = 128
B = 16
E = 8
TOPK = 2
NCORES = 8
PAIRS_PER_CORE = 4
BATCH_PER_CORE = 2

_compiled = None


def _build_program():
    import concourse.bass as bass
    import concourse.mybir as mybir
    from concourse import bacc
    from concourse.tile import TileContext

    f32 = mybir.dt.float32
    f32r = mybir.dt.float32r
    bf16 = mybir.dt.bfloat16
    GELU = mybir.ActivationFunctionType.Gelu

    nc = bacc.Bacc("TRN2", target_bir_lowering=False, debug=False,
                   num_devices=NCORES)

    x2 = nc.declare_dram_parameter("x2", [BATCH_PER_CORE, C, H, W], bf16,
                                   isOutput=False)
    # weight layouts are k-major so the load DMA is contiguous
    w1T = nc.declare_dram_parameter("w1T", [128, PAIRS_PER_CORE, 3, 128], bf16,
                                    isOutput=False)
    w1sT = nc.declare_dram_parameter("w1sT", [128, PAIRS_PER_CORE, 3, 128],
                                     bf16, isOutput=False)
    w2T = nc.declare_dram_parameter("w2T", [128, PAIRS_PER_CORE, 9, 128], bf16,
                                    isOutput=False)
    y4 = nc.declare_dram_parameter("y4", [PAIRS_PER_CORE, C, H, W], f32,
                                   isOutput=True)

    NG = H // 8  # 16 groups; group g handles 4-row tiles at rows 4g and 64+4g

    # (dx, rhs col range, out col range): dx=0 first -> it opens the PSUM
    # accumulation group and covers all 128 columns; dx=+-1 taps accumulate
    # into a 127-column subrange (the missing column is the zero pad).
    DXS = [(0, 0, 128, 0), (-1, 0, 127, 1), (1, 1, 128, 0)]

    with TileContext(nc) as tc:
        with (
            tc.tile_pool(name="wpool", bufs=1) as wpool,
            tc.tile_pool(name="xpool", bufs=1) as xpool,
            tc.tile_pool(name="hpool", bufs=1) as hpool,
            tc.tile_pool(name="ypool", bufs=3) as ypool,
            tc.tile_pool(name="pspool", bufs=2, space="PSUM") as pspool,
        ):
            # --- weights resident in SBUF for the whole kernel ---
            w1t = wpool.tile([128, PAIRS_PER_CORE, 3, 128], bf16, name="w1t")
            w1st = wpool.tile([128, PAIRS_PER_CORE, 3, 128], bf16, name="w1st")
            w2t = wpool.tile([128, PAIRS_PER_CORE, 9, 128], bf16, name="w2t")
            # w1t first (first matmul needs it), then batch-0 x below, then
            # the rest of the weights -- keeps the critical path short.
            nc.sync.dma_start(out=w1t, in_=w1T[:])

            # --- H-padded image buffers (no W padding; rows contiguous) ---
            # lo (parts 0:64):  row slot i = x row i-1 (slots 0,129 zero)
            # hi (parts 64:128): row slot i = x row i   (slots 128,129 zero)
            xbufs = [xpool.tile([128, H + 2, W], bf16, name=f"xbuf{i}",
                                tag=f"xbuf{i}") for i in range(BATCH_PER_CORE)]
            # hbuf: row slot i = h row i-1 (slots 0,129 zero)
            hbuf = hpool.tile([128, H + 2, W], bf16, name="hbuf")

            # zero pad rows (interiors fully rewritten per batch/pair,
            # pads stay zero afterwards)
            def pad_and_load_x(bi, dma_lo, dma_hi):
                xb = xbufs[bi]
                nc.vector.memset(xb[0:64, 0, :], 0.0)
                nc.vector.memset(xb[0:64, H + 1, :], 0.0)
                nc.vector.memset(xb[64:128, H, :], 0.0)
                nc.vector.memset(xb[64:128, H + 1, :], 0.0)
                # chunk by x-row quarters, ordered so the first A-tile rows
                # (0:32) and first B-tile rows (64:96) land before the rest.
                # Each hi chunk byte-overlaps its lo chunk, so its wait
                # head-of-line-blocks the issuing stream -- put lo and hi on
                # DIFFERENT streams so all lo chunks trigger back-to-back.
                order = (0, 4, 1, 5, 2, 6, 3, 7)
                for k in order:
                    r0, r1 = 16 * k, 16 * (k + 1)
                    dma_lo.dma_start(out=xb[0:64, r0 + 1:r1 + 1, :],
                                     in_=x2[bi][:, r0:r1, :])
                for k in order:
                    r0, r1 = 16 * k, 16 * (k + 1)
                    dma_hi.dma_start(out=xb[64:128, r0:r1, :],
                                     in_=x2[bi][:, r0:r1, :])

            nc.vector.memset(hbuf[:, 0, :], 0.0)
            nc.vector.memset(hbuf[:, H + 1, :], 0.0)

            # batch 0 now (sync/HWDGE: starts immediately); batch 1 is
            # issued at the top of pair 1 below, on the gpsimd stream, so
            # it overlaps pair-0/1 compute without competing at startup.
            pad_and_load_x(0, nc.sync, nc.gpsimd)

            # warm the PE (HAM clock gate) with throwaway matmuls while the
            # x load streams in; they only depend on the (tiny) w1t DMA.
            warm = pspool.tile([128, 3, 128], f32, name="warm", tag="warm",
                               bufs=1)
            for _ in range(30):
                nc.tensor.matmul(warm, w1t[:, 0, 0, :], w1t[:, 0, :, :],
                                 start=True, stop=True)
            nc.sync.dma_start(out=w1st, in_=w1sT[:])
            nc.sync.dma_start(out=w2t, in_=w2T[:])

            # conv2 taps, dx=0 first (opens the accumulation group)
            TAPS = [(dy, dx, cl, ch, ol)
                    for dx, cl, ch, ol in DXS for dy in (-1, 0, 1)]

            for pair in range(PAIRS_PER_CORE):
                xbuf = xbufs[pair // 2]
                if pair == 1:
                    pad_and_load_x(1, nc.gpsimd, nc.gpsimd)

                # ---------------- conv1 + gelu ----------------
                for g in range(NG):
                    rA = 4 * g
                    rB = 64 + 4 * g
                    hA = pspool.tile([128, 4, 128], f32, name=f"hA_{pair}_{g}",
                                     tag="hA")
                    hB = pspool.tile([128, 4, 128], f32, name=f"hB_{pair}_{g}",
                                     tag="hB")
                    # packed K=128: taps (dy=-1 from lo) + (dy=0 from hi)
                    for i, (dx, cl, ch, ol) in enumerate(DXS):
                        n = ch - cl
                        nc.tensor.matmul(hA[:, :, ol:ol + n],
                                         w1t[:, pair, i, :],
                                         xbuf[:, rA:rA + 4, cl:ch],
                                         start=(i == 0), stop=False)
                    for i, (dx, cl, ch, ol) in enumerate(DXS):
                        n = ch - cl
                        nc.tensor.matmul(hB[:, :, ol:ol + n],
                                         w1t[:, pair, i, :],
                                         xbuf[:, rB:rB + 4, cl:ch],
                                         start=(i == 0), stop=False)
                    # single tap dy=+1, K=64. A reads lo rows (row-groups
                    # 0-1), B reads hi rows (row-groups 2-3) -> concurrent.
                    for i, (dx, cl, ch, ol) in enumerate(DXS):
                        n = ch - cl
                        nc.tensor.matmul(hA[:, :, ol:ol + n],
                                         w1st[0:64, pair, i, :],
                                         xbuf[0:64, rA + 2:rA + 6, cl:ch],
                                         start=False, stop=(i == 2))
                        nc.tensor.matmul(hB[:, :, ol:ol + n],
                                         w1st[64:128, pair, i, :],
                                         xbuf[64:128, rB + 1:rB + 5, cl:ch],
                                         start=False, stop=(i == 2))
                    nc.scalar.activation(hbuf[:, rA + 1:rA + 5, :], hA, GELU)
                    nc.scalar.activation(hbuf[:, rB + 1:rB + 5, :], hB, GELU)

                # ---------------- conv2 (bf16, col-group concurrency) ------
                for g in range(NG):
                    rA = 4 * g
                    rB = 64 + 4 * g
                    yps = pspool.tile([128, 4, 128], f32,
                                      name=f"y_{pair}_{g}", tag="y")
                    for t, (dy, dx, cl, ch, ol) in enumerate(TAPS):
                        st = (t == 0)
                        sp = (t == 8)
                        n = ch - cl
                        # tile A -> psum parts 0:64 (col-group 0), tile B ->
                        # 64:128 (col-group 1): hardware-concurrent in bf16
                        nc.tensor.matmul(yps[0:64, :, ol:ol + n],
                                         w2t[:, pair, t, 0:64],
                                         hbuf[:, rA + 1 + dy:rA + 5 + dy,
                                              cl:ch],
                                         start=st, stop=sp)
                        nc.tensor.matmul(yps[64:128, :, ol:ol + n],
                                         w2t[:, pair, t, 64:128],
                                         hbuf[:, rB + 1 + dy:rB + 5 + dy,
                                              cl:ch],
                                         start=st, stop=sp)
                    ysb = ypool.tile([128, 4, 128], f32,
                                     name=f"ysb_{pair}_{g}", tag="ysb")
                    nc.vector.tensor_copy(ysb, yps)
                    nc.sync.dma_start(out=y4[pair][:, rA:rA + 4, :],
                                      in_=ysb[0:64])
                    nc.sync.dma_start(out=y4[pair][:, rB:rB + 4, :],
                                      in_=ysb[64:128])

    nc.compile()
    return nc


def _get_program():
    global _compiled
    if _compiled is None:
        _compiled = _build_program()
    return _compiled


def _cv_squared(v):
    v = v.astype(np.float64)
    if v.size <= 1:
        return 0.0
    return v.var(ddof=1) / (v.mean() ** 2 + 1e-10)


def _host_gating(x, gate_w, gate_b):
    """Mirror the reference gating exactly (fp32 math, top-2, softmax)."""
    avg = x.mean(axis=(2, 3), dtype=np.float32)
    mx = x.max(axis=(2, 3))
    xv = np.concatenate([avg, mx], axis=1).astype(np.float32)
    logits = (xv @ gate_w.T.astype(np.float32) + gate_b).astype(np.float32)
    top_idx = np.argsort(-logits, axis=1, kind="stable")[:, :TOPK]
    top_vals = np.take_along_axis(logits, top_idx, axis=1)
    m = top_vals.max(axis=1, keepdims=True)
    ex = np.exp((top_vals - m).astype(np.float32))
    top_gates = (ex / ex.sum(axis=1, keepdims=True)).astype(np.float32)
    gates = np.zeros((B, E), np.float32)
    gates[np.arange(B)[:, None], top_idx] = top_gates
    return gates, top_idx, top_gates


def kernel(x, w1, w2, gate_w, gate_b):
    import ml_dtypes
    from concourse.bass_utils import run_bass_kernel_spmd

    x = np.ascontiguousarray(x, dtype=np.float32)
    w1 = np.ascontiguousarray(w1, dtype=np.float32)
    w2 = np.ascontiguousarray(w2, dtype=np.float32)
    gate_w = np.ascontiguousarray(gate_w, dtype=np.float32)
    gate_b = np.ascontiguousarray(gate_b, dtype=np.float32)

    gates, top_idx, top_gates = _host_gating(x, gate_w, gate_b)

    importance = gates.sum(0)
    load = (gates > 0).sum(0).astype(np.float32)
    loss = np.float32((_cv_squared(importance) + _cv_squared(load)) * 0.01)

    # pair p = 2*b + j uses expert top_idx[b, j] with gate top_gates[b, j]
    experts = top_idx.reshape(-1)          # [32]
    gvals = top_gates.reshape(-1)          # [32]

    # conv1 weights, pre-transposed to [K, pair, dx, M] (k-major for DMA).
    # packed rows 0:64 = tap dy=-1, rows 64:128 = tap dy=0; singles = dy=+1
    # duplicated in both halves. dx slot order matches DXS: (0, -1, +1).
    dxorder = [1, 0, 2]  # dx index (0->dx=-1,1->dx=0,2->dx=+1) in DXS order
    w1sel = w1[experts]                    # [32, 128, 64, 3, 3]
    w1Tg = np.empty((B * TOPK, 128, 3, 128), np.float32)
    w1Tg[:, 0:64] = np.transpose(w1sel[:, :, :, 0, :], (0, 2, 3, 1))[:, :, dxorder]
    w1Tg[:, 64:128] = np.transpose(w1sel[:, :, :, 1, :], (0, 2, 3, 1))[:, :, dxorder]
    w1sTg = np.empty((B * TOPK, 128, 3, 128), np.float32)
    tap2 = np.transpose(w1sel[:, :, :, 2, :], (0, 2, 3, 1))[:, :, dxorder]
    w1sTg[:, 0:64] = tap2
    w1sTg[:, 64:128] = tap2

    # conv2 weights with gate folded in, duplicated along M for col-tiling.
    # tap order must match TAPS: dx in (0,-1,+1) outer, dy in (-1,0,1) inner.
    w2sel = w2[experts] * gvals[:, None, None, None, None]  # [32,64,128,3,3]
    w2r = np.transpose(w2sel, (0, 2, 4, 3, 1))  # [p, k, dx, dy, m]
    w2r = w2r[:, :, [1, 0, 2], :, :].reshape(B * TOPK, 128, 9, 64)
    w2Tg = np.concatenate([w2r, w2r], axis=3)   # [32, 128, 9, 128]

    nc = _get_program()
    in_maps = []
    for c in range(NCORES):
        p0 = PAIRS_PER_CORE * c
        sl = slice(p0, p0 + PAIRS_PER_CORE)
        in_maps.append({
            "x2": np.ascontiguousarray(
                x[BATCH_PER_CORE * c: BATCH_PER_CORE * (c + 1)]
            ).astype(ml_dtypes.bfloat16),
            "w1T": np.ascontiguousarray(
                w1Tg[sl].transpose(1, 0, 2, 3)).astype(ml_dtypes.bfloat16),
            "w1sT": np.ascontiguousarray(
                w1sTg[sl].transpose(1, 0, 2, 3)).astype(ml_dtypes.bfloat16),
            "w2T": np.ascontiguousarray(
                w2Tg[sl].transpose(1, 0, 2, 3)).astype(ml_dtypes.bfloat16),
        })

    res = run_bass_kernel_spmd(nc, in_maps, core_ids=list(range(NCORES)),
                               **_RUN_KWARGS)
    _LAST_RESULT[0] = res

    y = np.empty((B, C, H, W), np.float32)
    for c in range(NCORES):
        y4c = res.results[c]["y4"]
        y[2 * c] = y4c[0] + y4c[1]
        y[2 * c + 1] = y4c[2] + y4c[3]
    return y, loss


# test-harness knobs (kernel.py stays self-contained; defaults are no-ops)
_RUN_KWARGS = {}
_LAST_RESULT = [None]
